# revision 1
# baseline (speedup 1.0000x reference)
"""Bahdanau attention Trainium2 kernel (8 NeuronCores, batch-parallel).

Math (per batch b):
    e_projT[d, s] = sum_e W_e[e, d] * enc[b, s, e]            (PE, bf16)
    energyT[d, s] = tanh(e_projT + h_proj[b, d] + attn_b[d])  (ACT, bias per-partition)
    scores[s]     = sum_d v[d] * energyT[d, s]                (PE, M=1 matmuls)
    w = softmax(where(mask==0, -inf, scores))                 (row ops)
    context[e]    = sum_s w[s] * enc[b, s, e]                 (PE or DVE)

enc enters the PE with E2 on partitions (contraction dim), so each [128,128]
sub-tile of enc is transposed on-chip via PE transpose-mode matmuls (bf16,
1 cyc/row), packed 4-per-PSUM-bank, then copied to SBUF (DVE/ACT alternating).
A fraction of the transposes can instead ride the DMA xbar (SBUF->SBUF,
2-byte dtype) to offload the PE. All heavy matmuls run in bf16 (fp32 matmul
is 4x slower on TRN2); accumulation is fp32 in PSUM. h_proj is computed once
per core in fp32.
"""

import os
import sys

for _p in ("/opt/trn_rl_repo", "/root/.axon_site/_ro/trn_rl_repo"):
    if os.path.isdir(_p) and _p not in sys.path:
        sys.path.insert(0, _p)

# recover cleanly if a previous session left a core wedged
os.environ.setdefault("NEURON_RT_RESET_CORES", "1")

import numpy as np

import concourse.bass as bass
import concourse.tile as tile
from concourse import bacc, masks, mybir
from concourse._compat import with_exitstack
from concourse.bass_utils import run_bass_kernel_spmd

F32 = mybir.dt.float32
BF16 = mybir.dt.bfloat16
I32 = mybir.dt.int32
AF = mybir.ActivationFunctionType

N_CORES = 8
B, S, E2, D = 64, 1024, 1024, 512
BPC = B // N_CORES  # batches per core
NEG_INF = float(np.finfo(np.float32).min)

ST = S // 128   # 8 s-tiles per batch
ET = E2 // 128  # 8 e-blocks
DT = D // 128   # 4 d-tiles
NBLK = S // 512  # 2 s-halves of 512

# context on "pe" (nat-resident matmuls) or "dve" (wrep + mult/reduce)
CTX_MODE = os.environ.get("K_CTX_MODE", "pe")
# how many of the ET=8 e-blocks' transposes go via DMA xbar instead of PE
T_DMA_BLOCKS = int(os.environ.get("K_T_DMA_BLOCKS", "0"))
# repeat the whole per-core workload R times inside the NEFF (timing harness)
REPEAT = int(os.environ.get("K_REPEAT", "1"))


@with_exitstack
def _attn_kernel(ctx, tc, enc_d, hid_d, mask_d, w_d, b_d, v_d, ctx_d, aw_d):
    nc = tc.nc

    const = ctx.enter_context(tc.tile_pool(name="const", bufs=1))
    natp = ctx.enter_context(tc.tile_pool(name="nat", bufs=3))
    encTp = ctx.enter_context(tc.tile_pool(name="encT", bufs=2))
    enp = ctx.enter_context(tc.tile_pool(name="energy", bufs=10))
    smp = ctx.enter_context(tc.tile_pool(name="small", bufs=6))
    cxp = ctx.enter_context(tc.tile_pool(name="ctxc", bufs=3))

    # ---------------- constants / prologue ----------------
    ident_b = const.tile([128, 128], BF16)
    ident_f = const.tile([128, 128], F32)
    masks.make_identity(nc, ident_b[:])
    masks.make_identity(nc, ident_f[:])

    neginf = const.tile([1, S], F32)
    nc.gpsimd.memset(neginf[:], NEG_INF)
    ones_b = const.tile([1, 128], BF16)
    nc.gpsimd.memset(ones_b[:], 1.0)

    # prefetch the first batch's enc ahead of the bulk weight loads: the
    # transposes need only ident_b + nat, so the PE can start early.
    prenat = {}
    for b in range(min(1, BPC)):
        pn = natp.tile([128, ST * E2], BF16, tag="nat", name=f"nat_pre{b}")
        for t in range(ST):
            nc.gpsimd.dma_start(
                pn[:, t * E2 : (t + 1) * E2], enc_d[b, t * 128 : (t + 1) * 128, :]
            )
        prenat[b] = pn

    # weights as [p, k, d]: row index = k*128+p (2D contiguous DMA per k-slice)
    Wh = const.tile([128, DT, D], F32)
    for k in range(DT):
        nc.sync.dma_start(Wh[:, k, :], w_d[k * 128 : (k + 1) * 128, :])
    We = const.tile([128, ET, D], BF16)
    for k in range(ET):
        nc.gpsimd.dma_start(We[:, k, :], w_d[D + k * 128 : D + (k + 1) * 128, :])

    ab = const.tile([128, DT], F32)  # attn_b as [p, m], d = m*128+p
    nc.sync.dma_start(ab[:], b_d[:].rearrange("(m p) -> p m", p=128))
    vf = const.tile([128, DT], F32)  # v_W as [p, m]
    nc.sync.dma_start(vf[:], v_d[:].rearrange("(m p) -> p m", p=128))
    vb = const.tile([128, DT], BF16)
    nc.vector.tensor_copy(vb[:], vf[:])

    hid = const.tile([BPC, D], F32)
    nc.sync.dma_start(hid[:], hid_d[:])
    mask_t = const.tile([1, BPC * S], I32)
    nc.sync.dma_start(mask_t[:], mask_d[:])

    # hiddenT and h_projT + bias -> hb[p, m, b]
    hb = const.tile([128, DT, BPC], F32)
    hT = const.tile([128, DT, BPC], F32)
    with tc.tile_pool(name="psum_pro", bufs=2, space="PSUM") as psum_pro:
        for k in range(DT):
            tp = psum_pro.tile([128, BPC], F32, tag="pro")
            nc.tensor.transpose(tp[:], hid[:, k * 128 : (k + 1) * 128], ident_f[0:BPC, 0:BPC])
            nc.vector.tensor_copy(hT[:, k, :], tp[:])
        for m in range(DT):
            hp = psum_pro.tile([128, BPC], F32, tag="pro")
            for k in range(DT):
                nc.tensor.matmul(
                    hp[:],
                    Wh[:, k, m * 128 : (m + 1) * 128],
                    hT[:, k, :],
                    start=(k == 0),
                    stop=(k == DT - 1),
                )
            nc.vector.tensor_scalar(
                out=hb[:, m, :], in0=hp[:], scalar1=ab[:, m : m + 1], scalar2=None,
                op0=mybir.AluOpType.add,
            )

    # ---------------- main psum pools ----------------
    psum_t = ctx.enter_context(tc.tile_pool(name="psum_t", bufs=3, space="PSUM"))
    psum_mm = ctx.enter_context(tc.tile_pool(name="psum_mm", bufs=3, space="PSUM"))
    psum_sc = ctx.enter_context(tc.tile_pool(name="psum_sc", bufs=1, space="PSUM"))

    if REPEAT > 1:
        rep_cm = tc.For_i(0, REPEAT)
        rep_cm.__enter__()

    # software pipeline: batch b's context phase (which waits on b's softmax)
    # is emitted after batch b+1's transposes + e_proj, so the PE never
    # stalls on the softmax chain in program order.
    pending_ctx = []

    def emit_ctx(state):
        b, aw, wb, nat, encT = state
        if CTX_MODE == "pe":
            # w as columns [p, t]: 8 tiny PE transposes of [1,128] segments
            wcp = psum_sc.tile([128, ST], F32, tag="cps")
            for t in range(ST):
                nc.tensor.transpose(
                    wcp[:, t : t + 1], aw[:, t * 128 : (t + 1) * 128], ident_f[0:1, 0:1]
                )
            wcol = cxp.tile([128, ST], BF16, tag="wcol")
            nc.vector.tensor_copy(wcol[:], wcp[:])
            # contextT[1, e] = sum_t w_col[:,t].T @ nat[s-tile t, e-range]
            ctx_row = cxp.tile([1, E2], F32, tag="ctx_row")
            for half in range(2):
                cps = psum_sc.tile([1, 512], F32, tag="cps")
                for t in range(ST):
                    nc.tensor.matmul(
                        cps[:],
                        wcol[:, t : t + 1],
                        nat[:, t * E2 + half * 512 : t * E2 + half * 512 + 512],
                        start=(t == 0),
                        stop=(t == ST - 1),
                    )
                nc.vector.tensor_copy(ctx_row[:, half * 512 : (half + 1) * 512], cps[:])
            nc.sync.dma_start(ctx_d[b : b + 1, :], ctx_row[:])
        else:
            wrep = cxp.tile([128, S], BF16, tag="wrep")
            for blk in range(NBLK):
                wp = psum_mm.tile([128, 512], F32, tag="mm")
                nc.tensor.matmul(wp[:], ones_b[:], wb[:, blk * 512 : (blk + 1) * 512])
                nc.vector.tensor_copy(wrep[:, blk * 512 : (blk + 1) * 512], wp[:])
            ctx_cols = cxp.tile([128, ET], F32, tag="ctx_cols")
            for j in range(ET):
                junk = cxp.tile([128, S], BF16, tag="junk")
                nc.vector.tensor_tensor(
                    out=junk[:], in0=encT[:, j * S : (j + 1) * S], in1=wrep[:],
                    op=mybir.AluOpType.mult,
                )
                nc.vector.reduce_sum(
                    ctx_cols[:, j : j + 1], junk[:], axis=mybir.AxisListType.X
                )
            cps2 = psum_sc.tile([ET, 128], F32, tag="cps2")
            nc.tensor.transpose(cps2[:], ctx_cols[:], ident_f[:])
            ctx_row2 = cxp.tile([ET, 128], F32, tag="ctx_row2")
            nc.vector.tensor_copy(ctx_row2[:], cps2[:])
            nc.sync.dma_start(ctx_d[b].rearrange("(p f) -> p f", p=ET), ctx_row2[:])

    for b in range(BPC):
        # load enc[b] with cast to bf16: nat[p, t*E2 + e] = enc[b, t*128+p, e]
        if b in prenat:
            nat = prenat[b]
        else:
            nat = natp.tile([128, ST * E2], BF16, tag="nat")
            for t in range(ST):
                nc.gpsimd.dma_start(
                    nat[:, t * E2 : (t + 1) * E2], enc_d[b, t * 128 : (t + 1) * 128, :]
                )

        # transpose to encT[p, j*S + s] = enc[b, s, j*128+p]
        encT = encTp.tile([128, ET * S], BF16)
        cp_eng = 0
        for j in range(ET):
            if j < T_DMA_BLOCKS:
                # DMA xbar transpose, one [128s,128e] -> [128e,128s] tile at a time
                for t in range(ST):
                    nc.sync.dma_start(
                        encT[:, j * S + t * 128 : j * S + (t + 1) * 128],
                        nat[:, t * E2 + j * 128 : t * E2 + (j + 1) * 128],
                        transpose=True,
                    )
                continue
            for h in range(NBLK):
                tp = psum_t.tile([128, 512], BF16, tag="tp")
                for q in range(4):
                    t = 4 * h + q
                    nc.tensor.transpose(
                        tp[:, q * 128 : (q + 1) * 128],
                        nat[:, t * E2 + j * 128 : t * E2 + (j + 1) * 128],
                        ident_b[:],
                    )
                dst = encT[:, j * S + h * 512 : j * S + (h + 1) * 512]
                if cp_eng == 0:
                    nc.vector.tensor_copy(dst, tp[:])
                else:
                    nc.scalar.copy(dst, tp[:])
                cp_eng ^= 1

        # e_projT -> tanh (all blocks first, then scores: gives ACT slack)
        ens = {}
        for blk in range(NBLK):
            for m in range(DT):
                mm = psum_mm.tile([128, 512], F32, tag="mm")
                for k in range(ET):
                    nc.tensor.matmul(
                        mm[:],
                        We[:, k, m * 128 : (m + 1) * 128],
                        encT[:, k * S + blk * 512 : k * S + blk * 512 + 512],
                        start=(k == 0),
                        stop=(k == ET - 1),
                    )
                en = enp.tile([128, 512], BF16, tag="en")
                nc.scalar.activation(en[:], mm[:], AF.Tanh, bias=hb[:, m, b : b + 1])
                ens[(blk, m)] = en
        scores = smp.tile([1, S], F32, tag="row_f32")
        for blk in range(NBLK):
            sc = psum_sc.tile([1, 512], F32, tag="sc")
            for m in range(DT):
                nc.tensor.matmul(
                    sc[:], vb[:, m : m + 1], ens[(blk, m)][:],
                    start=(m == 0), stop=(m == DT - 1),
                )
            nc.vector.tensor_copy(scores[:, blk * 512 : (blk + 1) * 512], sc[:])

        # mask: out = where(mask != 0, scores, -inf)
        masked = smp.tile([1, S], F32, tag="row_f32")
        nc.vector.tensor_copy(masked[:], neginf[:])
        nc.vector.copy_predicated(masked[:], mask_t[:, b * S : (b + 1) * S], scores[:])

        # softmax on the [1, S] row
        nmax = smp.tile([1, 1], F32, tag="one")
        nc.vector.reduce_max(nmax[:], masked[:], axis=mybir.AxisListType.X, negate=True)
        expw = smp.tile([1, S], F32, tag="row_f32")
        sume = smp.tile([1, 1], F32, tag="one")
        nc.scalar.activation(expw[:], masked[:], AF.Exp, bias=nmax[:, 0:1], accum_out=sume[:])
        rcp = smp.tile([1, 1], F32, tag="one")
        nc.vector.reciprocal(rcp[:], sume[:])
        aw = smp.tile([1, S], F32, tag="row_f32")
        nc.vector.tensor_scalar_mul(aw[:], expw[:], rcp[:, 0:1])
        nc.sync.dma_start(aw_d[b : b + 1, :], aw[:])

        wb = smp.tile([1, S], BF16, tag="row_bf16")
        nc.vector.tensor_copy(wb[:], aw[:])

        pending_ctx.append((b, aw, wb, nat, encT))
        if len(pending_ctx) > 1:
            emit_ctx(pending_ctx.pop(0))

    while pending_ctx:
        emit_ctx(pending_ctx.pop(0))

    if REPEAT > 1:
        rep_cm.__exit__(None, None, None)


def build():
    nc = bacc.Bacc("TRN2", target_bir_lowering=False, debug=False)
    enc_d = nc.dram_tensor("enc", [BPC, S, E2], F32, kind="ExternalInput")
    hid_d = nc.dram_tensor("hidden", [BPC, D], F32, kind="ExternalInput")
    mask_d = nc.dram_tensor("mask", [BPC, S], I32, kind="ExternalInput")
    w_d = nc.dram_tensor("attn_w", [E2 + D, D], F32, kind="ExternalInput")
    b_d = nc.dram_tensor("attn_b", [D], F32, kind="ExternalInput")
    v_d = nc.dram_tensor("v_w", [D], F32, kind="ExternalInput")
    ctx_d = nc.dram_tensor("ctx_out", [BPC, E2], F32, kind="ExternalOutput")
    aw_d = nc.dram_tensor("aw_out", [BPC, S], F32, kind="ExternalOutput")

    with tile.TileContext(nc) as tc:
        _attn_kernel(tc, enc_d, hid_d, mask_d, w_d, b_d, v_d, ctx_d, aw_d)
    nc.compile()
    return nc


_NC_CACHE = None


def _get_nc():
    global _NC_CACHE
    if _NC_CACHE is None:
        _NC_CACHE = build()
    return _NC_CACHE


def _make_in_maps(hidden, encoder_outputs, mask, attn_W, attn_b, v_W):
    in_maps = []
    for c in range(N_CORES):
        sl = slice(c * BPC, (c + 1) * BPC)
        in_maps.append(
            {
                "enc": np.ascontiguousarray(encoder_outputs[sl]),
                "hidden": np.ascontiguousarray(hidden[sl]),
                "mask": np.ascontiguousarray(mask[sl]),
                "attn_w": attn_W,
                "attn_b": attn_b,
                "v_w": v_W,
            }
        )
    return in_maps


def run(hidden, encoder_outputs, mask, attn_W, attn_b, v_W, trace=False):
    """Run the bass kernel; returns ((context, attn_weights), BassKernelResults)."""
    nc = _get_nc()
    in_maps = _make_in_maps(hidden, encoder_outputs, mask, attn_W, attn_b, v_W)
    res = run_bass_kernel_spmd(nc, in_maps, list(range(N_CORES)), trace=trace)
    context = np.concatenate([res.results[c]["ctx_out"] for c in range(N_CORES)], axis=0)
    attn_w = np.concatenate([res.results[c]["aw_out"] for c in range(N_CORES)], axis=0)
    return (context, attn_w), res


def kernel(hidden, encoder_outputs, mask, attn_W, attn_b, v_W):
    (context, attn_w), _ = run(
        np.asarray(hidden, dtype=np.float32),
        np.asarray(encoder_outputs, dtype=np.float32),
        np.asarray(mask, dtype=np.int32),
        np.asarray(attn_W, dtype=np.float32),
        np.asarray(attn_b, dtype=np.float32),
        np.asarray(v_W, dtype=np.float32),
    )
    return context, attn_w


if __name__ == "__main__":
    nc = build()
    n_inst = sum(len(bb.instructions) for f in nc.m.functions for bb in f.blocks)
    print("build OK, instructions:", n_inst)



# revision 32
# speedup vs baseline: 2.0422x; 2.0422x over previous
"""Bahdanau attention Trainium2 kernel (8 NeuronCores, batch-parallel).

Math (per batch b):
    e_projT[d, s] = sum_e W_e[e, d] * enc[b, s, e]            (PE, fp8e4 DoubleRow)
    energyT[d, s] = tanh(e_projT/128 + h_proj[b, d] + attn_b[d])  (ACT, fp8 out)
    scores[s]     = sum_d v[d] * energyT[d, s]                (PE, fp8e4 DoubleRow)
    w = softmax(where(mask==0, -inf, scores))                 (row ops, f32)
    context[e]    = sum_s w[s] * enc[b, s, e]                 (PE, bf16)

enc is DMA-loaded with cast f32->bf16 (one SWDGE cast-DMA per batch), then
each [128,128] sub-tile is transposed on-chip via PE transpose-mode matmuls
(bf16, 1 cyc/row) and copy-cast to fp8e4 on DVE/ACT/Pool round-robin. The
e_proj and scores matmuls run in fp8e4 with MatmulPerfMode.DoubleRow (0.5
cyc/row: two 128-deep k-tiles per instruction); W_e and v are pre-scaled by
128 into fp8 to stay clear of e4m3 subnormals, and the scale is undone in the
tanh (ACT scale=1/128) / the scores PSUM->SBUF copy. The context matmul stays
bf16 (fp8 enc there would cost ~3e-2 absmax-rel on ctx, over the 2e-2 gate).
Accumulation is always f32 in PSUM; h_proj is computed once per core in f32.
"""

import os
import sys

for _p in ("/opt/trn_rl_repo", "/root/.axon_site/_ro/trn_rl_repo"):
    if os.path.isdir(_p) and _p not in sys.path:
        sys.path.insert(0, _p)

# recover cleanly if a previous session left a core wedged
os.environ.setdefault("NEURON_RT_RESET_CORES", "1")

import numpy as np

import concourse.bass as bass
import concourse.tile as tile
from concourse import bacc, masks, mybir
from concourse._compat import with_exitstack
from concourse.bass_utils import run_bass_kernel_spmd

F32 = mybir.dt.float32
BF16 = mybir.dt.bfloat16
FP8 = mybir.dt.float8e4
I32 = mybir.dt.int32
AF = mybir.ActivationFunctionType
DR = mybir.MatmulPerfMode.DoubleRow

N_CORES = 8
B, S, E2, D = 64, 1024, 1024, 512
BPC = B // N_CORES  # batches per core
NEG_INF = float(np.finfo(np.float32).min)

ST = S // 128   # 8 s-tiles per batch
ET = E2 // 128  # 8 e-blocks
DT = D // 128   # 4 d-tiles
NBLK = S // 512  # 2 s-halves of 512

WSCALE = 128.0  # fp8 pre-scale for W_e and v (undone after the matmuls)
XBAR = int(os.environ.get("K_XBAR", "0"))  # e-blocks transposed via DMA xbar

# repeat the whole per-core workload R times inside the NEFF (timing harness)
REPEAT = int(os.environ.get("K_REPEAT", "1"))
# timing experiment: hoist enc loads out of the repeat loop (stale data!)
NOLOAD = int(os.environ.get("K_NOLOAD", "0"))


@with_exitstack
def _attn_kernel(ctx, tc, enc_d, hid_d, mask_d, w_d, b_d, v_d, ctx_d, aw_d):
    nc = tc.nc

    const = ctx.enter_context(tc.tile_pool(name="const", bufs=1))
    natp = ctx.enter_context(tc.tile_pool(name="nat", bufs=5))
    encTp = ctx.enter_context(tc.tile_pool(name="encT", bufs=2))
    enp = ctx.enter_context(tc.tile_pool(name="energy", bufs=5))
    smp = ctx.enter_context(tc.tile_pool(name="small", bufs=6))
    cxp = ctx.enter_context(tc.tile_pool(name="ctxc", bufs=4))

    # ---------------- constants / prologue ----------------
    ident_b = const.tile([128, 128], BF16)
    ident_f = const.tile([128, 128], F32)
    masks.make_identity(nc, ident_b[:])
    masks.make_identity(nc, ident_f[:])

    neginfT = const.tile([128, ST], F32)
    nc.gpsimd.memset(neginfT[:], NEG_INF)
    ones_col = const.tile([128, 1], F32)
    nc.gpsimd.memset(ones_col[:], 1.0)
    ones_row = const.tile([1, 128], F32)
    nc.gpsimd.memset(ones_row[:], 1.0)

    # prefetch the first batch's enc ahead of the bulk weight loads: the
    # transposes need only ident_b + nat, so the PE can start early.
    prenat = {}
    for b in range(2 if NOLOAD else min(1, BPC)):
        pn = natp.tile([128, ST, E2], BF16, tag="nat", name=f"nat_pre{b}")
        nc.gpsimd.dma_start(pn[:], enc_d[b].rearrange("(t p) e -> p t e", p=128))
        prenat[b] = pn

    # small/critical sync DMAs first: hid feeds the prologue hb matmuls
    hid = const.tile([BPC, D], F32)
    nc.sync.dma_start(hid[:], hid_d[:])
    ab = const.tile([128, DT], F32)  # attn_b as [p, m], d = m*128+p
    nc.sync.dma_start(ab[:], b_d[:].rearrange("(m p) -> p m", p=128))
    vf = const.tile([128, DT], F32)  # v_W as [p, m]
    nc.sync.dma_start(vf[:], v_d[:].rearrange("(m p) -> p m", p=128))

    # weights as [p, k, d]: row index = k*128+p (2D contiguous DMA per k-slice)
    Wh = const.tile([128, DT, D], F32)
    for k in range(DT):
        nc.sync.dma_start(Wh[:, k, :], w_d[k * 128 : (k + 1) * 128, :])
    Web = const.tile([128, ET, D], BF16)
    nc.gpsimd.dma_start(
        Web[:], w_d[D:].rearrange("(k p) d -> p k d", p=128)
    )
    We8 = const.tile([128, ET, D], FP8)
    nc.gpsimd.tensor_scalar_mul(We8[:], Web[:], WSCALE)

    v8 = const.tile([128, DT, 1], FP8)
    nc.vector.tensor_scalar_mul(v8[:, :, 0], vf[:], WSCALE)

    # maskT[p, b*ST + t] = mask[b, t*128 + p]  (s-on-partitions layout);
    # first needed by softmax(0) during batch 1's bulk, so loaded late.
    maskT = const.tile([128, BPC * ST], I32)
    nc.sync.dma_start(maskT[:], mask_d[:].rearrange("b (t p) -> p (b t)", p=128))

    # hiddenT and h_projT + bias -> hb[p, m, b]
    hb = const.tile([128, DT, BPC], F32)
    hT = const.tile([128, DT, BPC], F32)
    with tc.tile_pool(name="psum_pro", bufs=2, space="PSUM") as psum_pro:
        for k in range(DT):
            tp = psum_pro.tile([128, BPC], F32, tag="pro")
            nc.tensor.transpose(tp[:], hid[:, k * 128 : (k + 1) * 128], ident_f[0:BPC, 0:BPC])
            nc.vector.tensor_copy(hT[:, k, :], tp[:])
        for m in range(DT):
            hp = psum_pro.tile([128, BPC], F32, tag="pro")
            for k in range(DT):
                nc.tensor.matmul(
                    hp[:],
                    Wh[:, k, m * 128 : (m + 1) * 128],
                    hT[:, k, :],
                    start=(k == 0),
                    stop=(k == DT - 1),
                )
            nc.vector.tensor_scalar(
                out=hb[:, m, :], in0=hp[:], scalar1=ab[:, m : m + 1], scalar2=None,
                op0=mybir.AluOpType.add,
            )

    # ---------------- main psum pools ----------------
    psum_t = ctx.enter_context(tc.tile_pool(name="psum_t", bufs=2, space="PSUM"))
    psum_mm = ctx.enter_context(tc.tile_pool(name="psum_mm", bufs=3, space="PSUM"))
    psum_sc = ctx.enter_context(tc.tile_pool(name="psum_sc", bufs=1, space="PSUM"))
    psum_x = ctx.enter_context(tc.tile_pool(name="psum_x", bufs=2, space="PSUM"))

    if REPEAT > 1:
        rep_cm = tc.For_i(0, REPEAT)
        rep_cm.__enter__()

    # software pipeline: batch b's context phase (which waits on b's softmax)
    # is emitted after batch b+1's transposes + e_proj, so the PE never
    # stalls on the softmax chain in program order.
    nats = dict(prenat)

    def load_nat(b):
        # load enc[b] with cast to bf16: nat[p, t, e] = enc[b, t*128+p, e]
        if b >= BPC or b in nats:
            return
        if NOLOAD:  # timing-only: reuse the two prologue tiles, no DMA
            nats[b] = nats[b % 2]
            return
        nat = natp.tile([128, ST, E2], BF16, tag="nat")
        nc.gpsimd.dma_start(nat[:], enc_d[b].rearrange("(t p) e -> p t e", p=128))
        nats[b] = nat

    def emit_bulk(b):
        """PE-heavy front half: transposes, e_proj+tanh, raw scores row."""
        nat = nats[b]
        # transpose to encT8[p, j, s] = enc[b, s, j*128+p], cast bf16 -> fp8.
        # e-blocks j < XBAR go through the DMA xbar transpose (SBUF->SBUF,
        # bf16) + a Pool cast to fp8 -- no PE cycles, no PSUM copy. The rest
        # ride PE transpose-mode matmuls + DVE/ACT copy-casts (Pool cannot
        # read PSUM).
        encT8 = encTp.tile([128, ET, S], FP8)
        if XBAR:
            encTb = encTp.tile([128, XBAR, S], BF16, tag="encTb", name=f"encTb{b}")
            for t in range(ST):
                nc.sync.dma_start(
                    encTb[:, :, t * 128 : (t + 1) * 128],
                    nat[:, t, 0 : XBAR * 128],
                    transpose=True,
                )
            for j in range(XBAR):
                nc.gpsimd.tensor_copy(encT8[:, j, :], encTb[:, j, :])
        copy_rot = [nc.vector, nc.scalar, nc.vector, nc.vector,
                    nc.scalar, nc.vector, nc.vector, nc.scalar]
        for j in range(XBAR, ET):
            tp = psum_t.tile([128, S], BF16, tag="tp")
            for t in range(ST):
                nc.tensor.transpose(
                    tp[:, t * 128 : (t + 1) * 128],
                    nat[:, t, j * 128 : (j + 1) * 128],
                    ident_b[:],
                )
            eng = copy_rot[j]
            if eng is nc.scalar:
                nc.scalar.copy(encT8[:, j, :], tp[:])
            else:
                eng.tensor_copy(encT8[:, j, :], tp[:])

        # e_projT (fp8 DoubleRow) -> tanh into fp8 energies [128, m, s-blk].
        # k outer over m-pairs: the first matmuls only need encT8 j-blocks
        # 0..1, so the PE starts before the later copies land.
        ens = {}
        for blk in range(NBLK):
            en = enp.tile([128, DT, 512], FP8, tag="en")
            for mp in range(DT // 2):
                mms = [psum_mm.tile([128, 512], F32, tag="mm", name=f"mm_{b}_{blk}_{mp}_{mi}")
                       for mi in range(2)]
                for k in range(ET // 2):
                    for mi in range(2):
                        m = 2 * mp + mi
                        nc.tensor.matmul(
                            mms[mi][:],
                            We8[:, 2 * k : 2 * k + 2, m * 128 : (m + 1) * 128],
                            encT8[:, 2 * k : 2 * k + 2, blk * 512 : blk * 512 + 512],
                            start=(k == 0),
                            stop=(k == ET // 2 - 1),
                            perf_mode=DR,
                        )
                for mi in range(2):
                    m = 2 * mp + mi
                    nc.scalar.activation(
                        en[:, m, :], mms[mi][:], AF.Tanh,
                        bias=hb[:, m, b : b + 1], scale=1.0 / WSCALE,
                    )
            ens[blk] = en
        return ens

    def emit_softmax(b, ens):
        """Raw scores (x128-scaled, s-on-partitions) + mask + softmax.
        Deferred one batch, so the tanh energies are long done and the
        serial chain latency hides under b+1's bulk.

        scoresT[s-chunk t, 1] = sum_m en8[:, m, chunk]^T @ v8[:, m] --
        classic fp8 matmuls with the energies as stationary: lands directly
        in the [128, ST] transposed layout (the M=1 DoubleRow form trips
        the s3_lw_dual_fp8 ISA restriction, and this saves the row
        transposes anyway)."""
        misc = psum_x.tile([128, 512], F32, tag="misc")
        scTp = misc[:, 0:ST]
        for t in range(ST):
            blk, c0 = t // 4, (t % 4) * 128
            for m in range(DT):
                nc.tensor.matmul(
                    scTp[:, t : t + 1],
                    ens[blk][:, m, c0 : c0 + 128],
                    v8[:, m, :],
                    start=(m == 0),
                    stop=(m == DT - 1),
                )

        # mask: out = where(mask != 0, scores, -inf)
        maskedT = smp.tile([128, ST], F32, tag="colT")
        nc.vector.tensor_copy(maskedT[:], neginfT[:])
        nc.vector.copy_predicated(maskedT[:], maskT[:, b * ST : (b + 1) * ST], scTp)

        # softmax, unnormalized exp first. scores are tiny (|s| <~ 6) so the
        # max-subtraction is unnecessary for f32 exp; masked lanes give
        # exp(-inf/128) = 0. ACT scale undoes the x128 weight pre-scale.
        expT = smp.tile([128, ST], F32, tag="colT")
        rowsum = smp.tile([128, 1], F32, tag="col1")
        nc.scalar.activation(
            expT[:], maskedT[:], AF.Exp, scale=1.0 / WSCALE, accum_out=rowsum[:]
        )
        # total = sum over partitions (PE), then 1/total broadcast to [128,1]
        totp = misc[0:1, ST : ST + 1]
        nc.tensor.matmul(totp, rowsum[:], ones_col[:], start=True, stop=True)
        tot = smp.tile([1, 1], F32, tag="one")
        nc.vector.tensor_copy(tot[:], totp)
        rcp = smp.tile([1, 1], F32, tag="one")
        nc.vector.reciprocal(rcp[:], tot[:])
        rcpp = misc[:, ST + 1 : ST + 2]
        nc.tensor.matmul(rcpp, ones_row[:], rcp[:], start=True, stop=True)
        rcpb = smp.tile([128, 1], F32, tag="col1")
        nc.vector.tensor_copy(rcpb[:], rcpp)

        awT = smp.tile([128, ST], F32, tag="colT")
        nc.vector.tensor_scalar_mul(awT[:], expT[:], rcpb[:, 0:1])
        nc.sync.dma_start(aw_d[b].rearrange("(t p) -> p t", p=128), awT[:])
        wcol = cxp.tile([128, ST], BF16, tag="wcol")
        nc.vector.tensor_copy(wcol[:], awT[:])
        return wcol

    def emit_ctx(b, wcol):
        """contextT[1, e] = sum_t w_col[:,t].T @ nat[s-tile t, e-range].
        Deferred two batches behind the bulk phase."""
        nat = nats[b]
        ctx_row = cxp.tile([1, E2], F32, tag="ctx_row")
        for half in range(2):
            cpst = psum_x.tile([128, 512], F32, tag="misc", name=f"cps_{b}_{half}")
            cps = cpst[0:1, 0:512]
            for t in range(ST):
                nc.tensor.matmul(
                    cps[:],
                    wcol[:, t : t + 1],
                    nat[:, t, half * 512 : half * 512 + 512],
                    start=(t == 0),
                    stop=(t == ST - 1),
                )
            if half == 0:
                nc.vector.tensor_copy(ctx_row[:, 0:512], cps[:])
            else:
                nc.vector.tensor_copy(ctx_row[:, 512:1024], cps[:])
        nc.sync.dma_start(ctx_d[b : b + 1, :], ctx_row[:])

    score_rows = {}
    wcols = {}
    for b in range(BPC):
        load_nat(b)
        load_nat(b + 1)
        score_rows[b] = emit_bulk(b)
        if b >= 3:
            emit_ctx(b - 3, wcols.pop(b - 3))
        if b >= 1:
            wcols[b - 1] = emit_softmax(b - 1, score_rows.pop(b - 1))
    wcols[BPC - 1] = emit_softmax(BPC - 1, score_rows.pop(BPC - 1))
    for b in (BPC - 3, BPC - 2, BPC - 1):
        emit_ctx(b, wcols.pop(b))

    if REPEAT > 1:
        rep_cm.__exit__(None, None, None)


def build():
    nc = bacc.Bacc("TRN2", target_bir_lowering=False, debug=False)
    enc_d = nc.dram_tensor("enc", [BPC, S, E2], F32, kind="ExternalInput")
    hid_d = nc.dram_tensor("hidden", [BPC, D], F32, kind="ExternalInput")
    mask_d = nc.dram_tensor("mask", [BPC, S], I32, kind="ExternalInput")
    w_d = nc.dram_tensor("attn_w", [E2 + D, D], F32, kind="ExternalInput")
    b_d = nc.dram_tensor("attn_b", [D], F32, kind="ExternalInput")
    v_d = nc.dram_tensor("v_w", [D], F32, kind="ExternalInput")
    ctx_d = nc.dram_tensor("ctx_out", [BPC, E2], F32, kind="ExternalOutput")
    aw_d = nc.dram_tensor("aw_out", [BPC, S], F32, kind="ExternalOutput")

    with tile.TileContext(nc) as tc:
        _attn_kernel(tc, enc_d, hid_d, mask_d, w_d, b_d, v_d, ctx_d, aw_d)
    nc.compile()
    return nc


_NC_CACHE = None


def _get_nc():
    global _NC_CACHE
    if _NC_CACHE is None:
        _NC_CACHE = build()
    return _NC_CACHE


def _make_in_maps(hidden, encoder_outputs, mask, attn_W, attn_b, v_W):
    in_maps = []
    for c in range(N_CORES):
        sl = slice(c * BPC, (c + 1) * BPC)
        in_maps.append(
            {
                "enc": np.ascontiguousarray(encoder_outputs[sl]),
                "hidden": np.ascontiguousarray(hidden[sl]),
                "mask": np.ascontiguousarray(mask[sl]),
                "attn_w": attn_W,
                "attn_b": attn_b,
                "v_w": v_W,
            }
        )
    return in_maps


def run(hidden, encoder_outputs, mask, attn_W, attn_b, v_W, trace=False):
    """Run the bass kernel; returns ((context, attn_weights), BassKernelResults)."""
    nc = _get_nc()
    in_maps = _make_in_maps(hidden, encoder_outputs, mask, attn_W, attn_b, v_W)
    res = run_bass_kernel_spmd(nc, in_maps, list(range(N_CORES)), trace=trace)
    context = np.concatenate([res.results[c]["ctx_out"] for c in range(N_CORES)], axis=0)
    attn_w = np.concatenate([res.results[c]["aw_out"] for c in range(N_CORES)], axis=0)
    return (context, attn_w), res


def kernel(hidden, encoder_outputs, mask, attn_W, attn_b, v_W):
    (context, attn_w), _ = run(
        np.asarray(hidden, dtype=np.float32),
        np.asarray(encoder_outputs, dtype=np.float32),
        np.asarray(mask, dtype=np.int32),
        np.asarray(attn_W, dtype=np.float32),
        np.asarray(attn_b, dtype=np.float32),
        np.asarray(v_W, dtype=np.float32),
    )
    return context, attn_w


if __name__ == "__main__":
    nc = build()
    n_inst = sum(len(bb.instructions) for f in nc.m.functions for bb in f.blocks)
    print("build OK, instructions:", n_inst)
